# revision 13
# baseline (speedup 1.0000x reference)
"""BiGCNN message-passing kernel for 8 Trainium2 NeuronCores (Bass/Tile).

Strategy (edge/graph parallelism):
  - Sort edges by destination (pass 1: e_v, pass 2: e_u) and give core r all
    edges whose destination falls in its equal row-range of the output.
    Edge tiles of 128 are packed so no destination segment straddles a tile.
  - Per edge tile: gather the two endpoint rows (indirect DMA), run the
    edge MLP (LN -> +gathers -> relu -> LN -> W_join -> LN), then reduce
    same-destination rows with a one-hot selection matmul and scatter the
    compacted rows to a local aggregation buffer (indirect DMA with OOB
    skip for padding).
  - LN affine params are folded into the weight matrices on the host, so the
    device only computes the normalize (x - mu) * rstd.
  - The variable/constraint row transforms (v_t, c_t) are computed on each
    core's own row slice and AllGathered when the other side needs to gather
    arbitrary rows (pass 1 needs full v_t, pass 2 needs full c_t2).
  - Merge stage runs over the core's own destination rows; outputs are the
    per-core row slices, concatenated on the host.
"""
import sys

if "/opt/trn_rl_repo" not in sys.path:
    sys.path.insert(0, "/opt/trn_rl_repo")

import numpy as np

import concourse.bass as bass
import concourse.tile as tile
from concourse import mybir
from concourse.bass import IndirectOffsetOnAxis
from concourse.bass_utils import run_bass_kernel_spmd
from concourse.masks import make_identity

P = 128
NCORES = 8
GRP = 4  # 128-edge tiles per op group (one PSUM bank of [128, 512])
EPS = 1e-5
F32 = mybir.dt.float32
I32 = mybir.dt.int32


# ---------------------------------------------------------------------------
# Tile-exit drain workaround: this walrus build only accepts a couple of sync
# waits per instruction, but TileContext's exit drain gets every global-clock
# wait attached to it.  Spread them one-per-nop before the drain.
# ---------------------------------------------------------------------------
from concourse.vector_clock import ScopedClock  # noqa: E402


def _patched_drain_and_barrier(self, tick_clock, wait_clock):
    nc = self.nc
    probe = nc.sync.nop(nofuse=True)
    wait_clock.add_sem_waits(probe.ins, ScopedClock({None: tick_clock.global_clock}))
    si = probe.ins.sync_info
    waits = list(si.on_wait) if si is not None else []
    upds = list(si.on_update) if si is not None else []
    probe.ins.sync_info = mybir.SyncInfo(
        on_wait=waits[:1], on_update=upds if len(waits) <= 1 else []
    )
    for i, w in enumerate(waits[1:]):
        n = nc.sync.nop(nofuse=True)
        is_last = i == len(waits) - 2
        n.ins.sync_info = mybir.SyncInfo(on_wait=[w], on_update=upds if is_last else [])
    nc.sync.drain()

    nc.all_engine_barrier()
    assert self.sems is not None
    popped = nc._tile_sem_poison_stack.pop()
    assert popped is self._sem_poison
    nc.clear_and_free_semaphores(list(self.sems.allocated().values()))
    nc.all_engine_barrier()


tile.TileContext._drain_and_barrier = _patched_drain_and_barrier

_NOPN = [0]


def _split_sync_waits(nc, cap=1):
    """This walrus build accepts very few sync waits per instruction.  Hoist
    extra waits onto same-engine nop instructions inserted just before."""
    for bb in nc.main_func.blocks:
        il = bb.instructions
        new = []
        changed = False
        for ins in il:
            si = ins.sync_info
            if si is not None and len(si.on_wait) > cap:
                waits = list(si.on_wait)
                for w in waits[:-cap]:
                    _NOPN[0] += 1
                    n = mybir.InstNoOp(name=f"waitnop-{_NOPN[0]}", ins=[], outs=[])
                    n.engine = ins.engine
                    n.sync_info = mybir.SyncInfo(on_wait=[w], on_update=[])
                    new.append(n)
                ins.sync_info = mybir.SyncInfo(on_wait=waits[-cap:],
                                               on_update=list(si.on_update))
                changed = True
            new.append(ins)
        if changed:
            il.clear()
            il.extend(new)


# ---------------------------------------------------------------------------
# Host-side preparation
# ---------------------------------------------------------------------------
def _fold_weights(i):
    f = np.float32
    Wl = (i["g_var"][:, None] * i["W_left"]).astype(f)
    bl = (i["b_var"] @ i["W_left"] + i["b_left"]).astype(f)
    We = (i["g_edge"][:, None] * i["W_edge"]).astype(f)
    be = (i["b_edge"] @ i["W_edge"]).astype(f)
    Wr = (i["g_con"][:, None] * i["W_right"]).astype(f)
    br = (i["b_con"] @ i["W_right"]).astype(f)
    Wj = (i["g_join_ln"][:, None] * i["W_join"]).astype(f)
    bj = (i["b_join_ln"] @ i["W_join"] + i["b_join"]).astype(f)
    EMB = Wl.shape[0]
    Wmt = i["W_merge"][:EMB].astype(f)
    Wmb = (i["g_joint"][:, None] * i["W_merge"][EMB:]).astype(f)
    bm_cnt = (i["b_joint"] @ i["W_merge"][EMB:]).astype(f)  # scales with seg count
    bm = i["b_merge"].astype(f)
    return dict(Wl=Wl, bl=bl, We=We, be=be, Wr=Wr, br=br, Wj=Wj, bj=bj,
                Wmt=Wmt, Wmb=Wmb, bm_cnt=bm_cnt, bm=bm,
                g_merge=i["g_merge"].astype(f), b_merge_ln=i["b_merge_ln"].astype(f))


def _prep_pass(dest, other, NDST, ncores, grp):
    """Shard+sort edges by destination, pack 128-edge tiles with no segment
    straddling a tile.  Returns per-core packed arrays (all [128, T]-packed)
    plus segment counts per local destination row."""
    NE = dest.shape[0]
    assert NDST % ncores == 0
    nloc = NDST // ncores
    order = np.argsort(dest, kind="stable")
    dsorted = dest[order]
    bounds = np.searchsorted(dsorted, np.arange(ncores + 1) * nloc)

    per_core = []
    max_tiles = 0
    for r in range(ncores):
        lo, hi = bounds[r], bounds[r + 1]
        eids = order[lo:hi]
        d_loc = (dsorted[lo:hi] - r * nloc).astype(np.int64)
        # segment run lengths (d_loc sorted ascending)
        uniq, counts = np.unique(d_loc, return_counts=True)
        # greedy tile packing without splitting segments
        tiles = []  # each: list of (start, length, seg_localdest)
        cur = []
        cur_n = 0
        pos = 0
        for u, c in zip(uniq, counts):
            assert c <= P, f"segment with {c} edges exceeds one tile"
            if cur_n + c > P:
                tiles.append((cur, cur_n))
                cur, cur_n = [], 0
            cur.append((pos, int(c), int(u)))
            cur_n += int(c)
            pos += int(c)
        if cur_n:
            tiles.append((cur, cur_n))
        per_core.append((eids, d_loc, tiles))
        max_tiles = max(max_tiles, len(tiles))

    T = -(-max_tiles // grp) * grp  # round up to group multiple
    SENT = nloc  # > bounds_check (nloc-1)  -> scatter skipped

    e_rows = np.full((ncores, T * P), -1, np.int64)
    g_other = np.zeros((ncores, T * P), np.int32)
    g_dloc = np.zeros((ncores, T * P), np.int32)
    lid = np.full((ncores, T * P), P - 1, np.float32)
    uid = np.full((ncores, T, P), SENT, np.int32)
    cnts = np.zeros((ncores, nloc), np.float32)

    for r in range(ncores):
        eids, d_loc, tiles = per_core[r]
        ocore = other[eids]
        np.add.at(cnts[r], d_loc, 1.0)
        for t, (segs, n) in enumerate(tiles):
            base = t * P
            slot = 0
            for u_i, (pos, c, u) in enumerate(segs):
                sl = slice(base + slot, base + slot + c)
                e_rows[r, sl] = eids[pos:pos + c]
                g_other[r, sl] = ocore[pos:pos + c]
                g_dloc[r, sl] = u
                lid[r, sl] = u_i
                uid[r, t, u_i] = u
                slot += c

    def pack(a):  # [ncores, T*P] -> [ncores, 128, T]
        return np.ascontiguousarray(a.reshape(ncores, T, P).transpose(0, 2, 1))

    return dict(T=T, nloc=nloc,
                e_rows=e_rows,
                g_other=pack(g_other), g_dloc=pack(g_dloc), lid=pack(lid),
                uid=np.ascontiguousarray(uid.transpose(0, 2, 1)),  # [nc,128,T]
                cnts=cnts)


# ---------------------------------------------------------------------------
# Device program
# ---------------------------------------------------------------------------
def _memset_dram(nc, zero_sb, dram_ap, n_rows):
    r0 = 0
    while r0 < n_rows:
        q = min(P, n_rows - r0)
        nc.sync.dma_start(out=dram_ap[r0:r0 + q, :], in_=zero_sb[:q, :P])
        r0 += q


def _ln_normalize(nc, sp, x_ap, z_ap, ngrp, rows=P, tag=""):
    """LayerNorm normalize (no affine): z = (x - mu) * rsqrt(var + eps).
    x_ap/z_ap are [rows, ngrp, 128] APs."""
    st = sp.tile([P, ngrp, 6], F32, tag=f"st{tag}")
    ag = sp.tile([P, ngrp, 2], F32, tag=f"ag{tag}")
    for g in range(ngrp):
        nc.vector.bn_stats(st[:rows, g, :],
                           x_ap[:, g, :] if ngrp > 1 else x_ap[:, 0, :])
        nc.vector.bn_aggr(ag[:rows, g, :], st[:rows, g, :])
    veps = sp.tile([P, ngrp, 1], F32, tag=f"ve{tag}")
    nc.vector.tensor_scalar(veps[:rows], ag[:rows, :, 1:2], EPS, None,
                            op0=mybir.AluOpType.add)
    rvar = sp.tile([P, ngrp, 1], F32, tag=f"rv{tag}")
    nc.vector.reciprocal(rvar[:rows], veps[:rows])
    rstd = sp.tile([P, ngrp, 1], F32, tag=f"rs{tag}")
    nc.scalar.sqrt(rstd[:rows], rvar[:rows])
    for g in range(ngrp):
        nc.vector.tensor_scalar(
            z_ap[:, g, :] if ngrp > 1 else z_ap[:, 0, :],
            x_ap[:, g, :] if ngrp > 1 else x_ap[:, 0, :],
            ag[:rows, g, 0:1], rstd[:rows, g, 0:1],
            op0=mybir.AluOpType.subtract, op1=mybir.AluOpType.mult)
    return rstd


def build_program(NU, NV, T1, T2, flags, grp=GRP):
    assert NU % NCORES == 0 and NV % NCORES == 0
    NUloc, NVloc = NU // NCORES, NV // NCORES

    nc = bass.Bass()
    dp = nc.declare_dram_parameter

    # --- parameters ---
    var_own = dp("var_own", [NUloc, P], F32, isOutput=False)
    con_own = dp("con_own", [NVloc, P], F32, isOutput=False)
    ze1 = dp("ze1", [T1 * P, P], F32, isOutput=False)
    ze2 = dp("ze2", [T2 * P, P], F32, isOutput=False)
    vg1 = dp("vg1", [P, T1], I32, isOutput=False)
    cg1 = dp("cg1", [P, T1], I32, isOutput=False)
    lid1 = dp("lid1", [P, T1], F32, isOutput=False)
    uid1 = dp("uid1", [P, T1], I32, isOutput=False)
    vg2 = dp("vg2", [P, T2], I32, isOutput=False)
    cg2 = dp("cg2", [P, T2], I32, isOutput=False)
    lid2 = dp("lid2", [P, T2], F32, isOutput=False)
    uid2 = dp("uid2", [P, T2], I32, isOutput=False)
    Wl_d = dp("Wl", [P, P], F32, isOutput=False)
    We_d = dp("We", [P, P], F32, isOutput=False)
    Wr_d = dp("Wr", [P, P], F32, isOutput=False)
    Wj_d = dp("Wj", [P, P], F32, isOutput=False)
    Wmt_d = dp("Wmt", [P, P], F32, isOutput=False)
    Wmb_d = dp("Wmb", [P, P], F32, isOutput=False)
    iota_d = dp("iota128", [P, P], F32, isOutput=False)
    # replicated const rows (used only when the matching flag is set):
    # 0:bl 1:be 2:br 3:bj 4:bm 5:g_merge 6:b_merge_ln 7:bm_cnt
    crows_d = dp("crows", [8, P, P], F32, isOutput=False)
    b2_d = dp("b2rows", [2, P], F32, isOutput=False)
    cnt1_d = dp("cnt1", [NVloc, 1], F32, isOutput=False)
    cnt2_d = dp("cnt2", [NUloc, 1], F32, isOutput=False)

    new_con = dp("new_con_own", [NVloc, P], F32, isOutput=True)
    new_var = dp("new_var_own", [NUloc, P], F32, isOutput=True)

    rg = [list(range(NCORES))]

    with tile.TileContext(nc) as tc:
        import contextlib
        with contextlib.ExitStack() as ctx:
            dram = ctx.enter_context(tc.tile_pool(name="dram", bufs=1, space="DRAM"))
            wp = ctx.enter_context(tc.tile_pool(name="wp", bufs=1))
            sp = ctx.enter_context(tc.tile_pool(name="sp", bufs=3))
            pp = ctx.enter_context(tc.tile_pool(name="pp", bufs=2, space="PSUM"))

            # internal DRAM
            vt_own = dram.tile([NUloc, P], F32)
            vt_full = dram.tile([NU, P], F32)
            ct1_own = dram.tile([NVloc, P], F32)
            agg1 = dram.tile([NVloc, P], F32)
            ct2_own = dram.tile([NVloc, P], F32)
            ct2_full = dram.tile([NV, P], F32)
            agg2 = dram.tile([NUloc, P], F32)

            # --- phase 0: persistent SBUF ---
            def load_w(d):
                t = wp.tile([P, P], F32, name=f"w_{d.name}")
                nc.sync.dma_start(t[:], d[:])
                return t

            Wl_s, We_s, Wr_s, Wj_s, Wmt_s, Wmb_s, iota_s = map(
                load_w, [Wl_d, We_d, Wr_d, Wj_d, Wmt_d, Wmb_d, iota_d])
            crow_s = {}
            for k, idx in (("bl", 0), ("be", 1), ("br", 2), ("bj", 3), ("bm", 4),
                           ("g_merge", 5), ("b_merge_ln", 6), ("bm_cnt", 7)):
                if flags.get(k):
                    t = wp.tile([P, P], F32, name=f"crow_{k}")
                    nc.sync.dma_start(t[:], crows_d[idx])
                    crow_s[k] = t
            b2_s = None
            if flags.get("bm") or flags.get("bm_cnt"):
                b2_s = wp.tile([2, P], F32)
                nc.sync.dma_start(b2_s[:], b2_d[:])
            ident = wp.tile([P, P], F32)
            make_identity(nc, ident[:])
            zero_sb = wp.tile([P, 512], F32)
            nc.vector.memset(zero_sb[:], 0.0)

            def load_idx(d, T, dt):
                t = wp.tile([P, T], dt, name=f"idx_{d.name}")
                nc.sync.dma_start(t[:], d[:])
                return t

            vg1_s = load_idx(vg1, T1, I32)
            cg1_s = load_idx(cg1, T1, I32)
            lid1_s = load_idx(lid1, T1, F32)
            uid1_s = load_idx(uid1, T1, I32)
            vg2_s = load_idx(vg2, T2, I32)
            cg2_s = load_idx(cg2, T2, I32)
            lid2_s = load_idx(lid2, T2, F32)
            uid2_s = load_idx(uid2, T2, I32)

            _memset_dram(nc, zero_sb, agg1[:], NVloc)
            _memset_dram(nc, zero_sb, agg2[:], NUloc)

            # --- row transform: out_dram <- LN(in_dram) @ W (+ bias row) ---
            def row_transform(src, n_rows, W_s, bias_key, out_dram, tag):
                nt = -(-n_rows // P)
                for t in range(nt):
                    r0 = t * P
                    rows = min(P, n_rows - r0)
                    x = sp.tile([P, 1, P], F32, tag=f"x{tag}")
                    nc.sync.dma_start(x[:rows, 0, :], src[r0:r0 + rows, :])
                    z = sp.tile([P, 1, P], F32, tag=f"z{tag}")
                    _ln_normalize(nc, sp, x[:rows], z[:rows], 1, rows, tag=tag)
                    pzt = pp.tile([P, P], F32, tag="zt")
                    nc.tensor.transpose(out=pzt[:, :rows], in_=z[:rows, 0, :],
                                        identity=ident[:rows, :rows])
                    zt = sp.tile([P, P], F32, tag=f"zt{tag}")
                    nc.vector.tensor_copy(zt[:, :rows], pzt[:, :rows])
                    pv = pp.tile([P, P], F32, tag="s")
                    nc.tensor.matmul(out=pv[:rows], lhsT=zt[:, :rows], rhs=W_s[:],
                                     start=True, stop=True)
                    o = sp.tile([P, P], F32, tag=f"o{tag}")
                    if bias_key and flags.get(bias_key):
                        nc.vector.tensor_tensor(o[:rows], pv[:rows],
                                                crow_s[bias_key][:rows],
                                                op=mybir.AluOpType.add)
                    else:
                        nc.scalar.copy(o[:rows], pv[:rows])
                    nc.sync.dma_start(out_dram[r0:r0 + rows, :], o[:rows])

            # --- phase 1: v_t, c_t1 + AllGather v_t ---
            row_transform(var_own, NUloc, Wl_s, "bl", vt_own[:], "p1v")
            row_transform(con_own, NVloc, Wr_s, "br", ct1_own[:], "p1c")
            nc.gpsimd.collective_compute(
                "AllGather", mybir.AluOpType.bypass, replica_groups=rg,
                ins=[vt_own.opt()], outs=[vt_full.opt()])

            # --- stage A: edge pipeline over groups of GRP tiles ---
            def stage_a(T, ze_d, vtab, vidx_s, ctab, cidx_s, lid_s, uid_s,
                        aggD, nloc, tag):
                zev = ze_d[:].rearrange("(t p) f -> p t f", p=P)
                bc_reg = nc.gpsimd.to_reg(nloc - 1)
                bc = mybir.AluOpType
                for gi in range(T // grp):
                    t0 = gi * grp
                    ze = sp.tile([P, grp, P], F32, tag="A_ze")
                    nc.sync.dma_start(ze[:], zev[:, t0:t0 + grp, :])
                    vgt = sp.tile([P, grp, P], F32, tag="A_vg")
                    cgt = sp.tile([P, grp, P], F32, tag="A_cg")
                    for g in range(grp):
                        nc.gpsimd.indirect_dma_start(
                            out=vgt[:, g, :], out_offset=None, in_=vtab,
                            in_offset=IndirectOffsetOnAxis(
                                ap=vidx_s[:, t0 + g:t0 + g + 1], axis=0))
                        nc.gpsimd.indirect_dma_start(
                            out=cgt[:, g, :], out_offset=None, in_=ctab,
                            in_offset=IndirectOffsetOnAxis(
                                ap=cidx_s[:, t0 + g:t0 + g + 1], axis=0))
                    # LN1 on raw edge rows
                    z = sp.tile([P, grp, P], F32, tag="A_z")
                    _ln_normalize(nc, sp, ze[:], z[:], grp, tag="A1")
                    # transpose z -> zt
                    pzt = pp.tile([P, grp, P], F32, tag="zt")
                    for g in range(grp):
                        nc.tensor.transpose(out=pzt[:, g, :], in_=z[:, g, :],
                                            identity=ident[:])
                    zt = sp.tile([P, grp, P], F32, tag="A_zt")
                    nc.scalar.copy(zt[:], pzt[:])
                    # s = z @ We (+be) + vg + cg ; relu
                    ps = pp.tile([P, grp, P], F32, tag="s")
                    for g in range(grp):
                        nc.tensor.matmul(out=ps[:, g, :], lhsT=zt[:, g, :],
                                         rhs=We_s[:], start=True, stop=True)
                    s1 = sp.tile([P, grp, P], F32, tag="A_s1")
                    nc.vector.tensor_tensor(s1[:], ps[:], vgt[:], op=bc.add)
                    if flags.get("be"):
                        nc.vector.tensor_tensor(
                            s1[:], s1[:],
                            crow_s["be"][:].rearrange("p f -> p () f").to_broadcast([P, grp, P]),
                            op=bc.add)
                    r = sp.tile([P, grp, P], F32, tag="A_r")
                    nc.vector.tensor_tensor(r[:], s1[:], cgt[:], op=bc.add)
                    nc.scalar.activation(r[:], r[:],
                                         mybir.ActivationFunctionType.Relu)
                    # LN2
                    z2 = sp.tile([P, grp, P], F32, tag="A_z2")
                    _ln_normalize(nc, sp, r[:], z2[:], grp, tag="A2")
                    # j = z2 @ Wj (+bj)
                    pzt2 = pp.tile([P, grp, P], F32, tag="zt")
                    for g in range(grp):
                        nc.tensor.transpose(out=pzt2[:, g, :], in_=z2[:, g, :],
                                            identity=ident[:])
                    zt2 = sp.tile([P, grp, P], F32, tag="A_zt2")
                    nc.scalar.copy(zt2[:], pzt2[:])
                    pj = pp.tile([P, grp, P], F32, tag="j")
                    for g in range(grp):
                        nc.tensor.matmul(out=pj[:, g, :], lhsT=zt2[:, g, :],
                                         rhs=Wj_s[:], start=True, stop=True)
                    if flags.get("bj"):
                        jt = sp.tile([P, grp, P], F32, tag="A_jt")
                        nc.vector.tensor_tensor(
                            jt[:], pj[:],
                            crow_s["bj"][:].rearrange("p f -> p () f").to_broadcast([P, grp, P]),
                            op=bc.add)
                        j_ap = jt[:]
                    else:
                        j_ap = pj[:]
                    # LN3 (joint, affine folded into Wmb / cnt row)
                    z3 = sp.tile([P, grp, P], F32, tag="A_z3")
                    _ln_normalize(nc, sp, j_ap, z3[:], grp, tag="A3")
                    # selection matrix S_T[p, u] = (lid[p] == u)
                    st_t = sp.tile([P, grp, P], F32, tag="A_st")
                    nc.vector.tensor_tensor(
                        st_t[:],
                        lid_s[:, t0:t0 + grp].rearrange("p g -> p g ()").to_broadcast([P, grp, P]),
                        iota_s[:].rearrange("p f -> p () f").to_broadcast([P, grp, P]),
                        op=bc.is_equal)
                    pagg = pp.tile([P, grp, P], F32, tag="agg")
                    for g in range(grp):
                        nc.tensor.matmul(out=pagg[:, g, :], lhsT=st_t[:, g, :],
                                         rhs=z3[:, g, :], start=True, stop=True)
                    aggs = sp.tile([P, grp, P], F32, tag="A_aggs")
                    nc.scalar.copy(aggs[:], pagg[:])
                    for g in range(grp):
                        nc.gpsimd.indirect_dma_start(
                            out=aggD, out_offset=IndirectOffsetOnAxis(
                                ap=uid_s[:, t0 + g:t0 + g + 1], axis=0),
                            in_=aggs[:, g, :], in_offset=None,
                            bounds_check=bc_reg, oob_is_err=False)

            stage_a(T1, ze1, vt_full[:], vg1_s, ct1_own[:], cg1_s,
                    lid1_s, uid1_s, agg1[:], NVloc, "1")

            # --- stage B: merge + residual over own destination rows ---
            def stage_b(n_rows, h_dram, aggD, cnt_d, cnt_key, out_dram,
                        ct2_out, tag):
                bc = mybir.AluOpType
                nt = -(-n_rows // P)
                for t in range(nt):
                    r0 = t * P
                    rows = min(P, n_rows - r0)
                    h = sp.tile([P, P], F32, tag="B_h")
                    nc.sync.dma_start(h[:rows], h_dram[r0:r0 + rows, :])
                    a = sp.tile([P, P], F32, tag="B_a")
                    nc.sync.dma_start(a[:rows], aggD[r0:r0 + rows, :])
                    pzt = pp.tile([P, P], F32, tag="zt")
                    nc.tensor.transpose(out=pzt[:, :rows], in_=h[:rows],
                                        identity=ident[:rows, :rows])
                    ht = sp.tile([P, P], F32, tag="B_ht")
                    nc.vector.tensor_copy(ht[:, :rows], pzt[:, :rows])
                    pzt2 = pp.tile([P, P], F32, tag="zt")
                    nc.tensor.transpose(out=pzt2[:, :rows], in_=a[:rows],
                                        identity=ident[:rows, :rows])
                    at = sp.tile([P, P], F32, tag="B_at")
                    nc.vector.tensor_copy(at[:, :rows], pzt2[:, :rows])
                    pm = pp.tile([P, P], F32, tag="s")
                    nc.tensor.matmul(out=pm[:rows], lhsT=ht[:, :rows],
                                     rhs=Wmt_s[:], start=True, stop=False)
                    use_cnt = flags.get(cnt_key)
                    nc.tensor.matmul(out=pm[:rows], lhsT=at[:, :rows],
                                     rhs=Wmb_s[:], start=False,
                                     stop=not (use_cnt or flags.get("bm")))
                    if use_cnt or flags.get("bm"):
                        onec = sp.tile([P, 2], F32, tag="B_onec")
                        if use_cnt:
                            nc.sync.dma_start(onec[:rows, 0:1],
                                              cnt_d[r0:r0 + rows, :])
                        else:
                            nc.vector.memset(onec[:rows, 0:1], 0.0)
                        nc.vector.memset(onec[:rows, 1:2], 1.0)
                        # pm += cnt * bm_cnt_row + 1 * bm_row  (rank-2 update)
                        pzt3 = pp.tile([P, P], F32, tag="zt")
                        nc.tensor.transpose(out=pzt3[:2, :rows], in_=onec[:rows],
                                            identity=ident[:rows, :rows])
                        oct_ = sp.tile([2, P], F32, tag="B_oct")
                        nc.vector.tensor_copy(oct_[:2, :rows], pzt3[:2, :rows])
                        nc.tensor.matmul(out=pm[:rows], lhsT=oct_[:2, :rows],
                                         rhs=b2_s[:2, :], start=False, stop=True)
                    rr = sp.tile([P, 1, P], F32, tag="B_r")
                    nc.scalar.activation(rr[:rows, 0, :], pm[:rows],
                                         mybir.ActivationFunctionType.Relu)
                    zm = sp.tile([P, 1, P], F32, tag="B_zm")
                    _ln_normalize(nc, sp, rr[:rows], zm[:rows], 1, rows, tag="B")
                    za = zm[:rows, 0, :]
                    if flags.get("g_merge"):
                        nc.vector.tensor_tensor(za, za, crow_s["g_merge"][:rows],
                                                op=bc.mult)
                    if flags.get("b_merge_ln"):
                        nc.vector.tensor_tensor(za, za,
                                                crow_s["b_merge_ln"][:rows],
                                                op=bc.add)
                    outt = sp.tile([P, P], F32, tag="B_out")
                    nc.vector.tensor_tensor(outt[:rows], za, h[:rows], op=bc.add)
                    nc.sync.dma_start(out_dram[r0:r0 + rows, :], outt[:rows])
                    if ct2_out is not None:
                        z4 = sp.tile([P, 1, P], F32, tag="B_z4")
                        _ln_normalize(nc, sp, outt[:rows].rearrange("p f -> p () f"),
                                      z4[:rows], 1, rows, tag="B2")
                        pzt4 = pp.tile([P, P], F32, tag="zt")
                        nc.tensor.transpose(out=pzt4[:, :rows],
                                            in_=z4[:rows, 0, :],
                                            identity=ident[:rows, :rows])
                        zt4 = sp.tile([P, P], F32, tag="B_zt4")
                        nc.vector.tensor_copy(zt4[:, :rows], pzt4[:, :rows])
                        pc2 = pp.tile([P, P], F32, tag="s")
                        nc.tensor.matmul(out=pc2[:rows], lhsT=zt4[:, :rows],
                                         rhs=Wr_s[:], start=True, stop=True)
                        o2 = sp.tile([P, P], F32, tag="B_o2")
                        if flags.get("br"):
                            nc.vector.tensor_tensor(o2[:rows], pc2[:rows],
                                                    crow_s["br"][:rows],
                                                    op=bc.add)
                        else:
                            nc.scalar.copy(o2[:rows], pc2[:rows])
                        nc.sync.dma_start(ct2_out[r0:r0 + rows, :], o2[:rows])

            stage_b(NVloc, ct1_own[:], agg1[:], cnt1_d, "bm_cnt1", new_con,
                    ct2_own[:], "1")

            nc.gpsimd.collective_compute(
                "AllGather", mybir.AluOpType.bypass, replica_groups=rg,
                ins=[ct2_own.opt()], outs=[ct2_full.opt()])

            stage_a(T2, ze2, vt_own[:], vg2_s, ct2_full[:], cg2_s,
                    lid2_s, uid2_s, agg2[:], NUloc, "2")
            stage_b(NUloc, vt_own[:], agg2[:], cnt2_d, "bm_cnt2", new_var,
                    None, "2")

    _split_sync_waits(nc)
    return nc


# ---------------------------------------------------------------------------
# Entry point
# ---------------------------------------------------------------------------
def _run(inputs, ncores=NCORES):
    i = {k: np.asarray(v) for k, v in inputs.items()}
    NU, EMB = i["variable_emb"].shape
    NV = i["constraint_emb"].shape[0]
    assert EMB == P
    w = _fold_weights(i)
    flags = {
        "bl": np.any(w["bl"] != 0), "be": np.any(w["be"] != 0),
        "br": np.any(w["br"] != 0), "bj": np.any(w["bj"] != 0),
        "bm": np.any(w["bm"] != 0),
        "bm_cnt1": np.any(w["bm_cnt"] != 0), "bm_cnt2": np.any(w["bm_cnt"] != 0),
        "bm_cnt": np.any(w["bm_cnt"] != 0),
        "g_merge": np.any(w["g_merge"] != 1.0),
        "b_merge_ln": np.any(w["b_merge_ln"] != 0),
    }
    e_u = i["e_u"].astype(np.int64)
    e_v = i["e_v"].astype(np.int64)
    p1 = _prep_pass(e_v, e_u, NV, ncores, GRP)  # dest=constraints, gather v_t
    p2 = _prep_pass(e_u, e_v, NU, ncores, GRP)  # dest=variables, gather c_t2

    nc = build_program(NU, NV, p1["T"], p2["T"], flags)

    edge_emb = i["edge_emb"].astype(np.float32)
    NUloc, NVloc = NU // ncores, NV // ncores
    crows = np.zeros((8, P, P), np.float32)
    for idx, k in enumerate(["bl", "be", "br", "bj", "bm"]):
        crows[idx, :, :] = w[k][None, :]
    crows[5, :, :] = w["g_merge"][None, :]
    crows[6, :, :] = w["b_merge_ln"][None, :]
    crows[7, :, :] = w["bm_cnt"][None, :]
    iota128 = np.broadcast_to(np.arange(P, dtype=np.float32), (P, P)).copy()

    def ze_arr(prep, r):
        er = prep["e_rows"][r]
        out = np.zeros((er.shape[0], P), np.float32)
        m = er >= 0
        out[m] = edge_emb[er[m]]
        return out

    in_maps = []
    for r in range(ncores):
        in_maps.append({
            "var_own": i["variable_emb"][r * NUloc:(r + 1) * NUloc].astype(np.float32),
            "con_own": i["constraint_emb"][r * NVloc:(r + 1) * NVloc].astype(np.float32),
            "ze1": ze_arr(p1, r), "ze2": ze_arr(p2, r),
            "vg1": p1["g_other"][r], "cg1": p1["g_dloc"][r],
            "lid1": p1["lid"][r], "uid1": p1["uid"][r],
            "vg2": p2["g_dloc"][r], "cg2": p2["g_other"][r],
            "lid2": p2["lid"][r], "uid2": p2["uid"][r],
            "Wl": w["Wl"], "We": w["We"], "Wr": w["Wr"], "Wj": w["Wj"],
            "Wmt": w["Wmt"], "Wmb": w["Wmb"],
            "iota128": iota128, "crows": crows,
            "b2rows": np.stack([w["bm_cnt"], w["bm"]]),
            "cnt1": p1["cnts"][r][:, None], "cnt2": p2["cnts"][r][:, None],
        })

    res = run_bass_kernel_spmd(nc, in_maps, list(range(ncores)))
    new_var = np.concatenate([res.results[r]["new_var_own"] for r in range(ncores)])
    new_con = np.concatenate([res.results[r]["new_con_own"] for r in range(ncores)])
    return new_var, new_con


def kernel(**inputs):
    return _run(inputs)


# revision 14
# speedup vs baseline: 4.1019x; 4.1019x over previous
"""BiGCNN message-passing kernel for 8 Trainium2 NeuronCores (Bass/Tile).

Strategy (edge/graph parallelism):
  - Sort edges by destination (pass 1: e_v, pass 2: e_u) and give core r all
    edges whose destination falls in its equal row-range of the output.
    Edge tiles of 128 are packed so no destination segment straddles a tile.
  - Per edge tile: gather the two endpoint rows (indirect DMA), run the
    edge MLP (LN -> +gathers -> relu -> LN -> W_join -> LN), then reduce
    same-destination rows with a one-hot selection matmul and scatter the
    compacted rows to a local aggregation buffer (indirect DMA with OOB
    skip for padding).
  - LN affine params are folded into the weight matrices on the host, so the
    device only computes the normalize (x - mu) * rstd.
  - The variable/constraint row transforms (v_t, c_t) are computed on each
    core's own row slice and AllGathered when the other side needs to gather
    arbitrary rows (pass 1 needs full v_t, pass 2 needs full c_t2).
  - Merge stage runs over the core's own destination rows; outputs are the
    per-core row slices, concatenated on the host.
"""
import sys

if "/opt/trn_rl_repo" not in sys.path:
    sys.path.insert(0, "/opt/trn_rl_repo")

import numpy as np

import concourse.bass as bass
import concourse.tile as tile
from concourse import mybir
from concourse.bass import IndirectOffsetOnAxis
from concourse.bass_utils import run_bass_kernel_spmd
from concourse.masks import make_identity

P = 128
NCORES = 8
GRP = 4  # 128-edge tiles per op group (one PSUM bank of [128, 512])
EPS = 1e-5
F32 = mybir.dt.float32
I32 = mybir.dt.int32


# ---------------------------------------------------------------------------
# Tile-exit drain workaround: this walrus build only accepts a couple of sync
# waits per instruction, but TileContext's exit drain gets every global-clock
# wait attached to it.  Spread them one-per-nop before the drain.
# ---------------------------------------------------------------------------
from concourse.vector_clock import ScopedClock  # noqa: E402


def _patched_drain_and_barrier(self, tick_clock, wait_clock):
    nc = self.nc
    probe = nc.sync.nop(nofuse=True)
    wait_clock.add_sem_waits(probe.ins, ScopedClock({None: tick_clock.global_clock}))
    si = probe.ins.sync_info
    waits = list(si.on_wait) if si is not None else []
    upds = list(si.on_update) if si is not None else []
    probe.ins.sync_info = mybir.SyncInfo(
        on_wait=waits[:1], on_update=upds if len(waits) <= 1 else []
    )
    for i, w in enumerate(waits[1:]):
        n = nc.sync.nop(nofuse=True)
        is_last = i == len(waits) - 2
        n.ins.sync_info = mybir.SyncInfo(on_wait=[w], on_update=upds if is_last else [])
    nc.sync.drain()

    nc.all_engine_barrier()
    assert self.sems is not None
    popped = nc._tile_sem_poison_stack.pop()
    assert popped is self._sem_poison
    nc.clear_and_free_semaphores(list(self.sems.allocated().values()))
    nc.all_engine_barrier()


tile.TileContext._drain_and_barrier = _patched_drain_and_barrier

_NOPN = [0]


def _split_sync_waits(nc, cap=1):
    """This walrus build accepts very few sync waits per instruction.  Hoist
    extra waits onto same-engine nop instructions inserted just before."""
    for bb in nc.main_func.blocks:
        il = bb.instructions
        new = []
        changed = False
        for ins in il:
            si = ins.sync_info
            if si is not None and len(si.on_wait) > cap:
                waits = list(si.on_wait)
                for w in waits[:-cap]:
                    _NOPN[0] += 1
                    n = mybir.InstNoOp(name=f"waitnop-{_NOPN[0]}", ins=[], outs=[])
                    n.engine = ins.engine
                    n.sync_info = mybir.SyncInfo(on_wait=[w], on_update=[])
                    new.append(n)
                ins.sync_info = mybir.SyncInfo(on_wait=waits[-cap:],
                                               on_update=list(si.on_update))
                changed = True
            new.append(ins)
        if changed:
            il.clear()
            il.extend(new)


# ---------------------------------------------------------------------------
# Host-side preparation
# ---------------------------------------------------------------------------
def _fold_weights(i):
    f = np.float32
    Wl = (i["g_var"][:, None] * i["W_left"]).astype(f)
    bl = (i["b_var"] @ i["W_left"] + i["b_left"]).astype(f)
    We = (i["g_edge"][:, None] * i["W_edge"]).astype(f)
    be = (i["b_edge"] @ i["W_edge"]).astype(f)
    Wr = (i["g_con"][:, None] * i["W_right"]).astype(f)
    br = (i["b_con"] @ i["W_right"]).astype(f)
    Wj = (i["g_join_ln"][:, None] * i["W_join"]).astype(f)
    bj = (i["b_join_ln"] @ i["W_join"] + i["b_join"]).astype(f)
    EMB = Wl.shape[0]
    Wmt = i["W_merge"][:EMB].astype(f)
    Wmb = (i["g_joint"][:, None] * i["W_merge"][EMB:]).astype(f)
    bm_cnt = (i["b_joint"] @ i["W_merge"][EMB:]).astype(f)  # scales with seg count
    bm = i["b_merge"].astype(f)
    return dict(Wl=Wl, bl=bl, We=We, be=be, Wr=Wr, br=br, Wj=Wj, bj=bj,
                Wmt=Wmt, Wmb=Wmb, bm_cnt=bm_cnt, bm=bm,
                g_merge=i["g_merge"].astype(f), b_merge_ln=i["b_merge_ln"].astype(f))


def _prep_pass(dest, other, NDST, ncores, grp):
    """Shard+sort edges by destination, pack 128-edge tiles with no segment
    straddling a tile.  Returns per-core packed arrays (all [128, T]-packed)
    plus segment counts per local destination row."""
    NE = dest.shape[0]
    assert NDST % ncores == 0
    nloc = NDST // ncores
    order = np.argsort(dest, kind="stable")
    dsorted = dest[order]
    bounds = np.searchsorted(dsorted, np.arange(ncores + 1) * nloc)

    per_core = []
    max_tiles = 0
    for r in range(ncores):
        lo, hi = bounds[r], bounds[r + 1]
        eids = order[lo:hi]
        d_loc = (dsorted[lo:hi] - r * nloc).astype(np.int64)
        # segment run lengths (d_loc sorted ascending)
        uniq, counts = np.unique(d_loc, return_counts=True)
        # greedy tile packing without splitting segments
        tiles = []  # each: list of (start, length, seg_localdest)
        cur = []
        cur_n = 0
        pos = 0
        for u, c in zip(uniq, counts):
            assert c <= P, f"segment with {c} edges exceeds one tile"
            if cur_n + c > P:
                tiles.append((cur, cur_n))
                cur, cur_n = [], 0
            cur.append((pos, int(c), int(u)))
            cur_n += int(c)
            pos += int(c)
        if cur_n:
            tiles.append((cur, cur_n))
        per_core.append((eids, d_loc, tiles))
        max_tiles = max(max_tiles, len(tiles))

    T = -(-max_tiles // grp) * grp  # round up to group multiple
    SENT = nloc  # > bounds_check (nloc-1)  -> scatter skipped

    e_rows = np.full((ncores, T * P), -1, np.int64)
    g_other = np.zeros((ncores, T * P), np.int32)
    g_dloc = np.zeros((ncores, T * P), np.int32)
    lid = np.full((ncores, T * P), P - 1, np.float32)
    uid = np.full((ncores, T, P), SENT, np.int32)
    cnts = np.zeros((ncores, nloc), np.float32)

    for r in range(ncores):
        eids, d_loc, tiles = per_core[r]
        ocore = other[eids]
        np.add.at(cnts[r], d_loc, 1.0)
        for t, (segs, n) in enumerate(tiles):
            base = t * P
            slot = 0
            for u_i, (pos, c, u) in enumerate(segs):
                sl = slice(base + slot, base + slot + c)
                e_rows[r, sl] = eids[pos:pos + c]
                g_other[r, sl] = ocore[pos:pos + c]
                g_dloc[r, sl] = u
                lid[r, sl] = u_i
                uid[r, t, u_i] = u
                slot += c

    def pack(a):  # [ncores, T*P] -> [ncores, 128, T]
        return np.ascontiguousarray(a.reshape(ncores, T, P).transpose(0, 2, 1))

    return dict(T=T, nloc=nloc,
                e_rows=e_rows,
                g_other=pack(g_other), g_dloc=pack(g_dloc), lid=pack(lid),
                uid=np.ascontiguousarray(uid.transpose(0, 2, 1)),  # [nc,128,T]
                cnts=cnts)


# ---------------------------------------------------------------------------
# Device program
# ---------------------------------------------------------------------------
def _memset_dram(nc, zero_sb, dram_ap, n_rows):
    r0 = 0
    while r0 < n_rows:
        q = min(P, n_rows - r0)
        nc.sync.dma_start(out=dram_ap[r0:r0 + q, :], in_=zero_sb[:q, :P])
        r0 += q


def _ln_normalize(nc, sp, x_ap, z_ap, ngrp, rows=P, tag=""):
    """LayerNorm normalize (no affine): z = (x - mu) * rsqrt(var + eps).
    x_ap/z_ap are [rows, ngrp, 128] APs."""
    st = sp.tile([P, ngrp, 6], F32, tag=f"st{tag}")
    ag = sp.tile([P, ngrp, 2], F32, tag=f"ag{tag}")
    for g in range(ngrp):
        nc.vector.bn_stats(st[:rows, g, :],
                           x_ap[:, g, :] if ngrp > 1 else x_ap[:, 0, :])
        nc.vector.bn_aggr(ag[:rows, g, :], st[:rows, g, :])
    veps = sp.tile([P, ngrp, 1], F32, tag=f"ve{tag}")
    nc.vector.tensor_scalar(veps[:rows], ag[:rows, :, 1:2], EPS, None,
                            op0=mybir.AluOpType.add)
    rvar = sp.tile([P, ngrp, 1], F32, tag=f"rv{tag}")
    nc.vector.reciprocal(rvar[:rows], veps[:rows])
    rstd = sp.tile([P, ngrp, 1], F32, tag=f"rs{tag}")
    nc.scalar.sqrt(rstd[:rows], rvar[:rows])
    for g in range(ngrp):
        nc.vector.tensor_scalar(
            z_ap[:, g, :] if ngrp > 1 else z_ap[:, 0, :],
            x_ap[:, g, :] if ngrp > 1 else x_ap[:, 0, :],
            ag[:rows, g, 0:1], rstd[:rows, g, 0:1],
            op0=mybir.AluOpType.subtract, op1=mybir.AluOpType.mult)
    return rstd


def build_program(NU, NV, T1, T2, flags, grp=GRP):
    assert NU % NCORES == 0 and NV % NCORES == 0
    NUloc, NVloc = NU // NCORES, NV // NCORES

    nc = bass.Bass()
    dp = nc.declare_dram_parameter

    # --- parameters ---
    var_own = dp("var_own", [NUloc, P], F32, isOutput=False)
    con_own = dp("con_own", [NVloc, P], F32, isOutput=False)
    ze1 = dp("ze1", [T1 * P, P], F32, isOutput=False)
    ze2 = dp("ze2", [T2 * P, P], F32, isOutput=False)
    vg1 = dp("vg1", [P, T1], I32, isOutput=False)
    cg1 = dp("cg1", [P, T1], I32, isOutput=False)
    lid1 = dp("lid1", [P, T1], F32, isOutput=False)
    uid1 = dp("uid1", [P, T1], I32, isOutput=False)
    vg2 = dp("vg2", [P, T2], I32, isOutput=False)
    cg2 = dp("cg2", [P, T2], I32, isOutput=False)
    lid2 = dp("lid2", [P, T2], F32, isOutput=False)
    uid2 = dp("uid2", [P, T2], I32, isOutput=False)
    Wl_d = dp("Wl", [P, P], F32, isOutput=False)
    We_d = dp("We", [P, P], F32, isOutput=False)
    Wr_d = dp("Wr", [P, P], F32, isOutput=False)
    Wj_d = dp("Wj", [P, P], F32, isOutput=False)
    Wmt_d = dp("Wmt", [P, P], F32, isOutput=False)
    Wmb_d = dp("Wmb", [P, P], F32, isOutput=False)
    iota_d = dp("iota128", [P, P], F32, isOutput=False)
    # replicated const rows (used only when the matching flag is set):
    # 0:bl 1:be 2:br 3:bj 4:bm 5:g_merge 6:b_merge_ln 7:bm_cnt
    crows_d = dp("crows", [8, P, P], F32, isOutput=False)
    b2_d = dp("b2rows", [2, P], F32, isOutput=False)
    cnt1_d = dp("cnt1", [NVloc, 1], F32, isOutput=False)
    cnt2_d = dp("cnt2", [NUloc, 1], F32, isOutput=False)

    new_con = dp("new_con_own", [NVloc, P], F32, isOutput=True)
    new_var = dp("new_var_own", [NUloc, P], F32, isOutput=True)

    rg = [list(range(NCORES))]

    with tile.TileContext(nc) as tc:
        import contextlib
        with contextlib.ExitStack() as ctx:
            dram = ctx.enter_context(tc.tile_pool(name="dram", bufs=1, space="DRAM"))
            wp = ctx.enter_context(tc.tile_pool(name="wp", bufs=1))
            sp = ctx.enter_context(tc.tile_pool(name="sp", bufs=3))
            pp = ctx.enter_context(tc.tile_pool(name="pp", bufs=2, space="PSUM"))

            # internal DRAM
            vt_own = dram.tile([NUloc, P], F32)
            vt_full = dram.tile([NU, P], F32, addr_space="Shared")
            ct1_own = dram.tile([NVloc, P], F32)
            agg1 = dram.tile([NVloc, P], F32)
            ct2_own = dram.tile([NVloc, P], F32)
            ct2_full = dram.tile([NV, P], F32, addr_space="Shared")
            agg2 = dram.tile([NUloc, P], F32)

            # --- phase 0: persistent SBUF ---
            def load_w(d):
                t = wp.tile([P, P], F32, name=f"w_{d.name}")
                nc.sync.dma_start(t[:], d[:])
                return t

            Wl_s, We_s, Wr_s, Wj_s, Wmt_s, Wmb_s, iota_s = map(
                load_w, [Wl_d, We_d, Wr_d, Wj_d, Wmt_d, Wmb_d, iota_d])
            crow_s = {}
            for k, idx in (("bl", 0), ("be", 1), ("br", 2), ("bj", 3), ("bm", 4),
                           ("g_merge", 5), ("b_merge_ln", 6), ("bm_cnt", 7)):
                if flags.get(k):
                    t = wp.tile([P, P], F32, name=f"crow_{k}")
                    nc.sync.dma_start(t[:], crows_d[idx])
                    crow_s[k] = t
            b2_s = None
            if flags.get("bm") or flags.get("bm_cnt"):
                b2_s = wp.tile([2, P], F32)
                nc.sync.dma_start(b2_s[:], b2_d[:])
            ident = wp.tile([P, P], F32)
            make_identity(nc, ident[:])
            zero_sb = wp.tile([P, 512], F32)
            nc.vector.memset(zero_sb[:], 0.0)

            def load_idx(d, T, dt):
                t = wp.tile([P, T], dt, name=f"idx_{d.name}")
                nc.sync.dma_start(t[:], d[:])
                return t

            vg1_s = load_idx(vg1, T1, I32)
            cg1_s = load_idx(cg1, T1, I32)
            lid1_s = load_idx(lid1, T1, F32)
            uid1_s = load_idx(uid1, T1, I32)
            vg2_s = load_idx(vg2, T2, I32)
            cg2_s = load_idx(cg2, T2, I32)
            lid2_s = load_idx(lid2, T2, F32)
            uid2_s = load_idx(uid2, T2, I32)

            _memset_dram(nc, zero_sb, agg1[:], NVloc)
            _memset_dram(nc, zero_sb, agg2[:], NUloc)

            # --- row transform: out_dram <- LN(in_dram) @ W (+ bias row) ---
            def row_transform(src, n_rows, W_s, bias_key, out_dram, tag):
                nt = -(-n_rows // P)
                for t in range(nt):
                    r0 = t * P
                    rows = min(P, n_rows - r0)
                    x = sp.tile([P, 1, P], F32, tag=f"x{tag}")
                    nc.sync.dma_start(x[:rows, 0, :], src[r0:r0 + rows, :])
                    z = sp.tile([P, 1, P], F32, tag=f"z{tag}")
                    _ln_normalize(nc, sp, x[:rows], z[:rows], 1, rows, tag=tag)
                    pzt = pp.tile([P, P], F32, tag="zt")
                    nc.tensor.transpose(out=pzt[:, :rows], in_=z[:rows, 0, :],
                                        identity=ident[:rows, :rows])
                    zt = sp.tile([P, P], F32, tag=f"zt{tag}")
                    nc.vector.tensor_copy(zt[:, :rows], pzt[:, :rows])
                    pv = pp.tile([P, P], F32, tag="s")
                    nc.tensor.matmul(out=pv[:rows], lhsT=zt[:, :rows], rhs=W_s[:],
                                     start=True, stop=True)
                    o = sp.tile([P, P], F32, tag=f"o{tag}")
                    if bias_key and flags.get(bias_key):
                        nc.vector.tensor_tensor(o[:rows], pv[:rows],
                                                crow_s[bias_key][:rows],
                                                op=mybir.AluOpType.add)
                    else:
                        nc.scalar.copy(o[:rows], pv[:rows])
                    nc.sync.dma_start(out_dram[r0:r0 + rows, :], o[:rows])

            # --- phase 1: v_t, c_t1 + AllGather v_t ---
            row_transform(var_own, NUloc, Wl_s, "bl", vt_own[:], "p1v")
            row_transform(con_own, NVloc, Wr_s, "br", ct1_own[:], "p1c")
            nc.gpsimd.collective_compute(
                "AllGather", mybir.AluOpType.bypass, replica_groups=rg,
                ins=[vt_own.opt()], outs=[vt_full.opt()])

            # --- stage A: edge pipeline over groups of GRP tiles ---
            def stage_a(T, ze_d, vtab, vidx_s, ctab, cidx_s, lid_s, uid_s,
                        aggD, nloc, tag):
                zev = ze_d[:].rearrange("(t p) f -> p t f", p=P)
                bc_reg = nc.gpsimd.to_reg(nloc - 1)
                bc = mybir.AluOpType
                for gi in range(T // grp):
                    t0 = gi * grp
                    ze = sp.tile([P, grp, P], F32, tag="A_ze")
                    nc.sync.dma_start(ze[:], zev[:, t0:t0 + grp, :])
                    vgt = sp.tile([P, grp, P], F32, tag="A_vg")
                    cgt = sp.tile([P, grp, P], F32, tag="A_cg")
                    for g in range(grp):
                        nc.gpsimd.indirect_dma_start(
                            out=vgt[:, g, :], out_offset=None, in_=vtab,
                            in_offset=IndirectOffsetOnAxis(
                                ap=vidx_s[:, t0 + g:t0 + g + 1], axis=0))
                        nc.gpsimd.indirect_dma_start(
                            out=cgt[:, g, :], out_offset=None, in_=ctab,
                            in_offset=IndirectOffsetOnAxis(
                                ap=cidx_s[:, t0 + g:t0 + g + 1], axis=0))
                    # LN1 on raw edge rows
                    z = sp.tile([P, grp, P], F32, tag="A_z")
                    _ln_normalize(nc, sp, ze[:], z[:], grp, tag="A1")
                    # transpose z -> zt
                    pzt = pp.tile([P, grp, P], F32, tag="zt")
                    for g in range(grp):
                        nc.tensor.transpose(out=pzt[:, g, :], in_=z[:, g, :],
                                            identity=ident[:])
                    zt = sp.tile([P, grp, P], F32, tag="A_zt")
                    nc.scalar.copy(zt[:], pzt[:])
                    # s = z @ We (+be) + vg + cg ; relu
                    ps = pp.tile([P, grp, P], F32, tag="s")
                    for g in range(grp):
                        nc.tensor.matmul(out=ps[:, g, :], lhsT=zt[:, g, :],
                                         rhs=We_s[:], start=True, stop=True)
                    s1 = sp.tile([P, grp, P], F32, tag="A_s1")
                    nc.vector.tensor_tensor(s1[:], ps[:], vgt[:], op=bc.add)
                    if flags.get("be"):
                        nc.vector.tensor_tensor(
                            s1[:], s1[:],
                            crow_s["be"][:].rearrange("p f -> p () f").to_broadcast([P, grp, P]),
                            op=bc.add)
                    r = sp.tile([P, grp, P], F32, tag="A_r")
                    nc.vector.tensor_tensor(r[:], s1[:], cgt[:], op=bc.add)
                    nc.scalar.activation(r[:], r[:],
                                         mybir.ActivationFunctionType.Relu)
                    # LN2
                    z2 = sp.tile([P, grp, P], F32, tag="A_z2")
                    _ln_normalize(nc, sp, r[:], z2[:], grp, tag="A2")
                    # j = z2 @ Wj (+bj)
                    pzt2 = pp.tile([P, grp, P], F32, tag="zt")
                    for g in range(grp):
                        nc.tensor.transpose(out=pzt2[:, g, :], in_=z2[:, g, :],
                                            identity=ident[:])
                    zt2 = sp.tile([P, grp, P], F32, tag="A_zt2")
                    nc.scalar.copy(zt2[:], pzt2[:])
                    pj = pp.tile([P, grp, P], F32, tag="j")
                    for g in range(grp):
                        nc.tensor.matmul(out=pj[:, g, :], lhsT=zt2[:, g, :],
                                         rhs=Wj_s[:], start=True, stop=True)
                    if flags.get("bj"):
                        jt = sp.tile([P, grp, P], F32, tag="A_jt")
                        nc.vector.tensor_tensor(
                            jt[:], pj[:],
                            crow_s["bj"][:].rearrange("p f -> p () f").to_broadcast([P, grp, P]),
                            op=bc.add)
                        j_ap = jt[:]
                    else:
                        j_ap = pj[:]
                    # LN3 (joint, affine folded into Wmb / cnt row)
                    z3 = sp.tile([P, grp, P], F32, tag="A_z3")
                    _ln_normalize(nc, sp, j_ap, z3[:], grp, tag="A3")
                    # selection matrix S_T[p, u] = (lid[p] == u)
                    st_t = sp.tile([P, grp, P], F32, tag="A_st")
                    nc.vector.tensor_tensor(
                        st_t[:],
                        lid_s[:, t0:t0 + grp].rearrange("p g -> p g ()").to_broadcast([P, grp, P]),
                        iota_s[:].rearrange("p f -> p () f").to_broadcast([P, grp, P]),
                        op=bc.is_equal)
                    pagg = pp.tile([P, grp, P], F32, tag="agg")
                    for g in range(grp):
                        nc.tensor.matmul(out=pagg[:, g, :], lhsT=st_t[:, g, :],
                                         rhs=z3[:, g, :], start=True, stop=True)
                    aggs = sp.tile([P, grp, P], F32, tag="A_aggs")
                    nc.scalar.copy(aggs[:], pagg[:])
                    for g in range(grp):
                        nc.gpsimd.indirect_dma_start(
                            out=aggD, out_offset=IndirectOffsetOnAxis(
                                ap=uid_s[:, t0 + g:t0 + g + 1], axis=0),
                            in_=aggs[:, g, :], in_offset=None,
                            bounds_check=bc_reg, oob_is_err=False)

            stage_a(T1, ze1, vt_full[:], vg1_s, ct1_own[:], cg1_s,
                    lid1_s, uid1_s, agg1[:], NVloc, "1")

            # --- stage B: merge + residual over own destination rows ---
            def stage_b(n_rows, h_dram, aggD, cnt_d, cnt_key, out_dram,
                        ct2_out, tag):
                bc = mybir.AluOpType
                nt = -(-n_rows // P)
                for t in range(nt):
                    r0 = t * P
                    rows = min(P, n_rows - r0)
                    h = sp.tile([P, P], F32, tag="B_h")
                    nc.sync.dma_start(h[:rows], h_dram[r0:r0 + rows, :])
                    a = sp.tile([P, P], F32, tag="B_a")
                    nc.sync.dma_start(a[:rows], aggD[r0:r0 + rows, :])
                    pzt = pp.tile([P, P], F32, tag="zt")
                    nc.tensor.transpose(out=pzt[:, :rows], in_=h[:rows],
                                        identity=ident[:rows, :rows])
                    ht = sp.tile([P, P], F32, tag="B_ht")
                    nc.vector.tensor_copy(ht[:, :rows], pzt[:, :rows])
                    pzt2 = pp.tile([P, P], F32, tag="zt")
                    nc.tensor.transpose(out=pzt2[:, :rows], in_=a[:rows],
                                        identity=ident[:rows, :rows])
                    at = sp.tile([P, P], F32, tag="B_at")
                    nc.vector.tensor_copy(at[:, :rows], pzt2[:, :rows])
                    pm = pp.tile([P, P], F32, tag="s")
                    nc.tensor.matmul(out=pm[:rows], lhsT=ht[:, :rows],
                                     rhs=Wmt_s[:], start=True, stop=False)
                    use_cnt = flags.get(cnt_key)
                    nc.tensor.matmul(out=pm[:rows], lhsT=at[:, :rows],
                                     rhs=Wmb_s[:], start=False,
                                     stop=not (use_cnt or flags.get("bm")))
                    if use_cnt or flags.get("bm"):
                        onec = sp.tile([P, 2], F32, tag="B_onec")
                        if use_cnt:
                            nc.sync.dma_start(onec[:rows, 0:1],
                                              cnt_d[r0:r0 + rows, :])
                        else:
                            nc.vector.memset(onec[:rows, 0:1], 0.0)
                        nc.vector.memset(onec[:rows, 1:2], 1.0)
                        # pm += cnt * bm_cnt_row + 1 * bm_row  (rank-2 update)
                        pzt3 = pp.tile([P, P], F32, tag="zt")
                        nc.tensor.transpose(out=pzt3[:2, :rows], in_=onec[:rows],
                                            identity=ident[:rows, :rows])
                        oct_ = sp.tile([2, P], F32, tag="B_oct")
                        nc.vector.tensor_copy(oct_[:2, :rows], pzt3[:2, :rows])
                        nc.tensor.matmul(out=pm[:rows], lhsT=oct_[:2, :rows],
                                         rhs=b2_s[:2, :], start=False, stop=True)
                    rr = sp.tile([P, 1, P], F32, tag="B_r")
                    nc.scalar.activation(rr[:rows, 0, :], pm[:rows],
                                         mybir.ActivationFunctionType.Relu)
                    zm = sp.tile([P, 1, P], F32, tag="B_zm")
                    _ln_normalize(nc, sp, rr[:rows], zm[:rows], 1, rows, tag="B")
                    za = zm[:rows, 0, :]
                    if flags.get("g_merge"):
                        nc.vector.tensor_tensor(za, za, crow_s["g_merge"][:rows],
                                                op=bc.mult)
                    if flags.get("b_merge_ln"):
                        nc.vector.tensor_tensor(za, za,
                                                crow_s["b_merge_ln"][:rows],
                                                op=bc.add)
                    outt = sp.tile([P, P], F32, tag="B_out")
                    nc.vector.tensor_tensor(outt[:rows], za, h[:rows], op=bc.add)
                    nc.sync.dma_start(out_dram[r0:r0 + rows, :], outt[:rows])
                    if ct2_out is not None:
                        z4 = sp.tile([P, 1, P], F32, tag="B_z4")
                        _ln_normalize(nc, sp, outt[:rows].rearrange("p f -> p () f"),
                                      z4[:rows], 1, rows, tag="B2")
                        pzt4 = pp.tile([P, P], F32, tag="zt")
                        nc.tensor.transpose(out=pzt4[:, :rows],
                                            in_=z4[:rows, 0, :],
                                            identity=ident[:rows, :rows])
                        zt4 = sp.tile([P, P], F32, tag="B_zt4")
                        nc.vector.tensor_copy(zt4[:, :rows], pzt4[:, :rows])
                        pc2 = pp.tile([P, P], F32, tag="s")
                        nc.tensor.matmul(out=pc2[:rows], lhsT=zt4[:, :rows],
                                         rhs=Wr_s[:], start=True, stop=True)
                        o2 = sp.tile([P, P], F32, tag="B_o2")
                        if flags.get("br"):
                            nc.vector.tensor_tensor(o2[:rows], pc2[:rows],
                                                    crow_s["br"][:rows],
                                                    op=bc.add)
                        else:
                            nc.scalar.copy(o2[:rows], pc2[:rows])
                        nc.sync.dma_start(ct2_out[r0:r0 + rows, :], o2[:rows])

            stage_b(NVloc, ct1_own[:], agg1[:], cnt1_d, "bm_cnt1", new_con,
                    ct2_own[:], "1")

            nc.gpsimd.collective_compute(
                "AllGather", mybir.AluOpType.bypass, replica_groups=rg,
                ins=[ct2_own.opt()], outs=[ct2_full.opt()])

            stage_a(T2, ze2, vt_own[:], vg2_s, ct2_full[:], cg2_s,
                    lid2_s, uid2_s, agg2[:], NUloc, "2")
            stage_b(NUloc, vt_own[:], agg2[:], cnt2_d, "bm_cnt2", new_var,
                    None, "2")

    _split_sync_waits(nc)
    return nc


# ---------------------------------------------------------------------------
# Entry point
# ---------------------------------------------------------------------------
_CACHE = {}


def _run(inputs, ncores=NCORES):
    i = {k: np.asarray(v) for k, v in inputs.items()}
    NU, EMB = i["variable_emb"].shape
    NV = i["constraint_emb"].shape[0]
    assert EMB == P
    w = _fold_weights(i)
    flags = {
        "bl": np.any(w["bl"] != 0), "be": np.any(w["be"] != 0),
        "br": np.any(w["br"] != 0), "bj": np.any(w["bj"] != 0),
        "bm": np.any(w["bm"] != 0),
        "bm_cnt1": np.any(w["bm_cnt"] != 0), "bm_cnt2": np.any(w["bm_cnt"] != 0),
        "bm_cnt": np.any(w["bm_cnt"] != 0),
        "g_merge": np.any(w["g_merge"] != 1.0),
        "b_merge_ln": np.any(w["b_merge_ln"] != 0),
    }
    e_u = i["e_u"].astype(np.int64)
    e_v = i["e_v"].astype(np.int64)
    import hashlib
    key = hashlib.sha1(
        e_u.tobytes() + e_v.tobytes()
        + repr(sorted(flags.items())).encode()
        + repr((NU, NV, ncores)).encode()).hexdigest()
    if key in _CACHE:
        p1, p2, nc = _CACHE[key]
    else:
        p1 = _prep_pass(e_v, e_u, NV, ncores, GRP)  # dest=constraints, gather v_t
        p2 = _prep_pass(e_u, e_v, NU, ncores, GRP)  # dest=variables, gather c_t2
        nc = build_program(NU, NV, p1["T"], p2["T"], flags)
        _CACHE.clear()
        _CACHE[key] = (p1, p2, nc)

    edge_emb = i["edge_emb"].astype(np.float32)
    NUloc, NVloc = NU // ncores, NV // ncores
    crows = np.zeros((8, P, P), np.float32)
    for idx, k in enumerate(["bl", "be", "br", "bj", "bm"]):
        crows[idx, :, :] = w[k][None, :]
    crows[5, :, :] = w["g_merge"][None, :]
    crows[6, :, :] = w["b_merge_ln"][None, :]
    crows[7, :, :] = w["bm_cnt"][None, :]
    iota128 = np.broadcast_to(np.arange(P, dtype=np.float32), (P, P)).copy()

    def ze_arr(prep, r):
        er = prep["e_rows"][r]
        out = np.zeros((er.shape[0], P), np.float32)
        m = er >= 0
        out[m] = edge_emb[er[m]]
        return out

    in_maps = []
    for r in range(ncores):
        in_maps.append({
            "var_own": i["variable_emb"][r * NUloc:(r + 1) * NUloc].astype(np.float32),
            "con_own": i["constraint_emb"][r * NVloc:(r + 1) * NVloc].astype(np.float32),
            "ze1": ze_arr(p1, r), "ze2": ze_arr(p2, r),
            "vg1": p1["g_other"][r], "cg1": p1["g_dloc"][r],
            "lid1": p1["lid"][r], "uid1": p1["uid"][r],
            "vg2": p2["g_dloc"][r], "cg2": p2["g_other"][r],
            "lid2": p2["lid"][r], "uid2": p2["uid"][r],
            "Wl": w["Wl"], "We": w["We"], "Wr": w["Wr"], "Wj": w["Wj"],
            "Wmt": w["Wmt"], "Wmb": w["Wmb"],
            "iota128": iota128, "crows": crows,
            "b2rows": np.stack([w["bm_cnt"], w["bm"]]),
            "cnt1": p1["cnts"][r][:, None], "cnt2": p2["cnts"][r][:, None],
        })

    res = run_bass_kernel_spmd(nc, in_maps, list(range(ncores)))
    new_var = np.concatenate([res.results[r]["new_var_own"] for r in range(ncores)])
    new_con = np.concatenate([res.results[r]["new_con_own"] for r in range(ncores)])
    return new_var, new_con


def kernel(**inputs):
    return _run(inputs)


# revision 15
# speedup vs baseline: 73.8617x; 18.0067x over previous
"""BiGCNN message-passing kernel for 8 Trainium2 NeuronCores (Bass/Tile).

Strategy (edge/graph parallelism):
  - Sort edges by destination (pass 1: e_v, pass 2: e_u) and give core r all
    edges whose destination falls in its equal row-range of the output.
    Edge tiles of 128 are packed so no destination segment straddles a tile.
  - Per edge tile: gather the two endpoint rows (indirect DMA), run the
    edge MLP (LN -> +gathers -> relu -> LN -> W_join -> LN), then reduce
    same-destination rows with a one-hot selection matmul and scatter the
    compacted rows to a local aggregation buffer (indirect DMA with OOB
    skip for padding).
  - LN affine params are folded into the weight matrices on the host, so the
    device only computes the normalize (x - mu) * rstd.
  - The variable/constraint row transforms (v_t, c_t) are computed on each
    core's own row slice and AllGathered when the other side needs to gather
    arbitrary rows (pass 1 needs full v_t, pass 2 needs full c_t2).
  - Merge stage runs over the core's own destination rows; outputs are the
    per-core row slices, concatenated on the host.
"""
import sys

if "/opt/trn_rl_repo" not in sys.path:
    sys.path.insert(0, "/opt/trn_rl_repo")

import numpy as np

import concourse.bass as bass
import concourse.tile as tile
from concourse import mybir
from concourse.bass import IndirectOffsetOnAxis
from concourse.bass_utils import run_bass_kernel_spmd
from concourse.masks import make_identity

P = 128
NCORES = 8
GRP = 4  # 128-edge tiles per op group (one PSUM bank of [128, 512])
EPS = 1e-5
F32 = mybir.dt.float32
I32 = mybir.dt.int32


# ---------------------------------------------------------------------------
# Tile-exit drain workaround: this walrus build only accepts a couple of sync
# waits per instruction, but TileContext's exit drain gets every global-clock
# wait attached to it.  Spread them one-per-nop before the drain.
# ---------------------------------------------------------------------------
from concourse.vector_clock import ScopedClock  # noqa: E402


def _patched_drain_and_barrier(self, tick_clock, wait_clock):
    nc = self.nc
    probe = nc.sync.nop(nofuse=True)
    wait_clock.add_sem_waits(probe.ins, ScopedClock({None: tick_clock.global_clock}))
    si = probe.ins.sync_info
    waits = list(si.on_wait) if si is not None else []
    upds = list(si.on_update) if si is not None else []
    probe.ins.sync_info = mybir.SyncInfo(
        on_wait=waits[:1], on_update=upds if len(waits) <= 1 else []
    )
    for i, w in enumerate(waits[1:]):
        n = nc.sync.nop(nofuse=True)
        is_last = i == len(waits) - 2
        n.ins.sync_info = mybir.SyncInfo(on_wait=[w], on_update=upds if is_last else [])
    nc.sync.drain()

    nc.all_engine_barrier()
    assert self.sems is not None
    popped = nc._tile_sem_poison_stack.pop()
    assert popped is self._sem_poison
    nc.clear_and_free_semaphores(list(self.sems.allocated().values()))
    nc.all_engine_barrier()


tile.TileContext._drain_and_barrier = _patched_drain_and_barrier

_NOPN = [0]


def _split_sync_waits(nc, cap=1):
    """This walrus build accepts very few sync waits per instruction.  Hoist
    extra waits onto same-engine nop instructions inserted just before."""
    for bb in nc.main_func.blocks:
        il = bb.instructions
        new = []
        changed = False
        for ins in il:
            si = ins.sync_info
            if si is not None and len(si.on_wait) > cap:
                waits = list(si.on_wait)
                for w in waits[:-cap]:
                    _NOPN[0] += 1
                    n = mybir.InstNoOp(name=f"waitnop-{_NOPN[0]}", ins=[], outs=[])
                    n.engine = ins.engine
                    n.sync_info = mybir.SyncInfo(on_wait=[w], on_update=[])
                    new.append(n)
                ins.sync_info = mybir.SyncInfo(on_wait=waits[-cap:],
                                               on_update=list(si.on_update))
                changed = True
            new.append(ins)
        if changed:
            il.clear()
            il.extend(new)


# ---------------------------------------------------------------------------
# Host-side preparation
# ---------------------------------------------------------------------------
def _fold_weights(i):
    f = np.float32
    Wl = (i["g_var"][:, None] * i["W_left"]).astype(f)
    bl = (i["b_var"] @ i["W_left"] + i["b_left"]).astype(f)
    We = (i["g_edge"][:, None] * i["W_edge"]).astype(f)
    be = (i["b_edge"] @ i["W_edge"]).astype(f)
    Wr = (i["g_con"][:, None] * i["W_right"]).astype(f)
    br = (i["b_con"] @ i["W_right"]).astype(f)
    Wj = (i["g_join_ln"][:, None] * i["W_join"]).astype(f)
    bj = (i["b_join_ln"] @ i["W_join"] + i["b_join"]).astype(f)
    EMB = Wl.shape[0]
    Wmt = i["W_merge"][:EMB].astype(f)
    Wmb = (i["g_joint"][:, None] * i["W_merge"][EMB:]).astype(f)
    bm_cnt = (i["b_joint"] @ i["W_merge"][EMB:]).astype(f)  # scales with seg count
    bm = i["b_merge"].astype(f)
    return dict(Wl=Wl, bl=bl, We=We, be=be, Wr=Wr, br=br, Wj=Wj, bj=bj,
                Wmt=Wmt, Wmb=Wmb, bm_cnt=bm_cnt, bm=bm,
                g_merge=i["g_merge"].astype(f), b_merge_ln=i["b_merge_ln"].astype(f))


def _prep_pass(dest, other, NDST, ncores, grp):
    """Shard+sort edges by destination, pack 128-edge tiles with no segment
    straddling a tile.  Returns per-core packed arrays (all [128, T]-packed)
    plus segment counts per local destination row."""
    NE = dest.shape[0]
    assert NDST % ncores == 0
    nloc = NDST // ncores
    order = np.argsort(dest, kind="stable")
    dsorted = dest[order]
    bounds = np.searchsorted(dsorted, np.arange(ncores + 1) * nloc)

    per_core = []
    max_tiles = 0
    for r in range(ncores):
        lo, hi = bounds[r], bounds[r + 1]
        eids = order[lo:hi]
        d_loc = (dsorted[lo:hi] - r * nloc).astype(np.int64)
        # segment run lengths (d_loc sorted ascending)
        uniq, counts = np.unique(d_loc, return_counts=True)
        # greedy tile packing without splitting segments
        tiles = []  # each: list of (start, length, seg_localdest)
        cur = []
        cur_n = 0
        pos = 0
        for u, c in zip(uniq, counts):
            assert c <= P, f"segment with {c} edges exceeds one tile"
            if cur_n + c > P:
                tiles.append((cur, cur_n))
                cur, cur_n = [], 0
            cur.append((pos, int(c), int(u)))
            cur_n += int(c)
            pos += int(c)
        if cur_n:
            tiles.append((cur, cur_n))
        per_core.append((eids, d_loc, tiles))
        max_tiles = max(max_tiles, len(tiles))

    T = -(-max_tiles // grp) * grp  # round up to group multiple
    SENT = nloc  # > bounds_check (nloc-1)  -> scatter skipped

    e_rows = np.full((ncores, T * P), -1, np.int64)
    g_other = np.zeros((ncores, T * P), np.int32)
    g_dloc = np.zeros((ncores, T * P), np.int32)
    lid = np.full((ncores, T * P), P - 1, np.float32)
    uid = np.full((ncores, T, P), SENT, np.int32)
    cnts = np.zeros((ncores, nloc), np.float32)

    for r in range(ncores):
        eids, d_loc, tiles = per_core[r]
        ocore = other[eids]
        np.add.at(cnts[r], d_loc, 1.0)
        for t, (segs, n) in enumerate(tiles):
            base = t * P
            slot = 0
            for u_i, (pos, c, u) in enumerate(segs):
                sl = slice(base + slot, base + slot + c)
                e_rows[r, sl] = eids[pos:pos + c]
                g_other[r, sl] = ocore[pos:pos + c]
                g_dloc[r, sl] = u
                lid[r, sl] = u_i
                uid[r, t, u_i] = u
                slot += c

    def pack(a):  # [ncores, T*P] -> [ncores, 128, T]
        return np.ascontiguousarray(a.reshape(ncores, T, P).transpose(0, 2, 1))

    return dict(T=T, nloc=nloc,
                e_rows=e_rows,
                g_other=pack(g_other), g_dloc=pack(g_dloc), lid=pack(lid),
                uid=np.ascontiguousarray(uid.transpose(0, 2, 1)),  # [nc,128,T]
                cnts=cnts)


# ---------------------------------------------------------------------------
# Device program
# ---------------------------------------------------------------------------
def _memset_dram(nc, zero_sb, dram_ap, n_rows):
    r0 = 0
    while r0 < n_rows:
        q = min(P, n_rows - r0)
        nc.sync.dma_start(out=dram_ap[r0:r0 + q, :], in_=zero_sb[:q, :P])
        r0 += q


def _ln_normalize(nc, sp, x_ap, z_ap, ngrp, rows=P, tag=""):
    """LayerNorm normalize (no affine): z = (x - mu) * rsqrt(var + eps).
    x_ap/z_ap are [rows, ngrp, 128] APs."""
    st = sp.tile([P, ngrp, 6], F32, tag=f"st{tag}")
    ag = sp.tile([P, ngrp, 2], F32, tag=f"ag{tag}")
    for g in range(ngrp):
        nc.vector.bn_stats(st[:rows, g, :],
                           x_ap[:, g, :] if ngrp > 1 else x_ap[:, 0, :])
        nc.vector.bn_aggr(ag[:rows, g, :], st[:rows, g, :])
    veps = sp.tile([P, ngrp, 1], F32, tag=f"ve{tag}")
    nc.vector.tensor_scalar(veps[:rows], ag[:rows, :, 1:2], EPS, None,
                            op0=mybir.AluOpType.add)
    rvar = sp.tile([P, ngrp, 1], F32, tag=f"rv{tag}")
    nc.vector.reciprocal(rvar[:rows], veps[:rows])
    rstd = sp.tile([P, ngrp, 1], F32, tag=f"rs{tag}")
    nc.scalar.sqrt(rstd[:rows], rvar[:rows])
    for g in range(ngrp):
        nc.vector.tensor_scalar(
            z_ap[:, g, :] if ngrp > 1 else z_ap[:, 0, :],
            x_ap[:, g, :] if ngrp > 1 else x_ap[:, 0, :],
            ag[:rows, g, 0:1], rstd[:rows, g, 0:1],
            op0=mybir.AluOpType.subtract, op1=mybir.AluOpType.mult)
    return rstd


def build_program(NU, NV, T1, T2, flags, grp=GRP):
    import os
    _ablate = os.environ.get("KABLATE", "")
    assert NU % NCORES == 0 and NV % NCORES == 0
    NUloc, NVloc = NU // NCORES, NV // NCORES

    nc = bass.Bass()
    dp = nc.declare_dram_parameter

    # --- parameters ---
    var_own = dp("var_own", [NUloc, P], F32, isOutput=False)
    con_own = dp("con_own", [NVloc, P], F32, isOutput=False)
    ze1 = dp("ze1", [T1 * P, P], F32, isOutput=False)
    ze2 = dp("ze2", [T2 * P, P], F32, isOutput=False)
    vg1 = dp("vg1", [P, T1], I32, isOutput=False)
    cg1 = dp("cg1", [P, T1], I32, isOutput=False)
    lid1 = dp("lid1", [P, T1], F32, isOutput=False)
    uid1 = dp("uid1", [P, T1], I32, isOutput=False)
    vg2 = dp("vg2", [P, T2], I32, isOutput=False)
    cg2 = dp("cg2", [P, T2], I32, isOutput=False)
    lid2 = dp("lid2", [P, T2], F32, isOutput=False)
    uid2 = dp("uid2", [P, T2], I32, isOutput=False)
    Wl_d = dp("Wl", [P, P], F32, isOutput=False)
    We_d = dp("We", [P, P], F32, isOutput=False)
    Wr_d = dp("Wr", [P, P], F32, isOutput=False)
    Wj_d = dp("Wj", [P, P], F32, isOutput=False)
    Wmt_d = dp("Wmt", [P, P], F32, isOutput=False)
    Wmb_d = dp("Wmb", [P, P], F32, isOutput=False)
    iota_d = dp("iota128", [P, P], F32, isOutput=False)
    # replicated const rows (used only when the matching flag is set):
    # 0:bl 1:be 2:br 3:bj 4:bm 5:g_merge 6:b_merge_ln 7:bm_cnt
    crows_d = dp("crows", [8, P, P], F32, isOutput=False)
    b2_d = dp("b2rows", [2, P], F32, isOutput=False)
    cnt1_d = dp("cnt1", [NVloc, 1], F32, isOutput=False)
    cnt2_d = dp("cnt2", [NUloc, 1], F32, isOutput=False)

    new_con = dp("new_con_own", [NVloc, P], F32, isOutput=True)
    new_var = dp("new_var_own", [NUloc, P], F32, isOutput=True)

    rg = [list(range(NCORES))]

    with tile.TileContext(nc) as tc:
        import contextlib
        with contextlib.ExitStack() as ctx:
            dram = ctx.enter_context(tc.tile_pool(name="dram", bufs=1, space="DRAM"))
            wp = ctx.enter_context(tc.tile_pool(name="wp", bufs=1))
            sp = ctx.enter_context(tc.tile_pool(name="sp", bufs=3))
            pp = ctx.enter_context(tc.tile_pool(name="pp", bufs=2, space="PSUM"))

            # internal DRAM
            vt_own = dram.tile([NUloc, P], F32)
            vt_full = dram.tile([NU, P], F32, addr_space="Shared")
            ct1_own = dram.tile([NVloc, P], F32)
            agg1 = dram.tile([NVloc, P], F32)
            ct2_own = dram.tile([NVloc, P], F32)
            ct2_full = dram.tile([NV, P], F32, addr_space="Shared")
            agg2 = dram.tile([NUloc, P], F32)

            # --- phase 0: persistent SBUF ---
            def load_w(d):
                t = wp.tile([P, P], F32, name=f"w_{d.name}")
                nc.sync.dma_start(t[:], d[:])
                return t

            Wl_s, We_s, Wr_s, Wj_s, Wmt_s, Wmb_s, iota_s = map(
                load_w, [Wl_d, We_d, Wr_d, Wj_d, Wmt_d, Wmb_d, iota_d])
            crow_s = {}
            for k, idx in (("bl", 0), ("be", 1), ("br", 2), ("bj", 3), ("bm", 4),
                           ("g_merge", 5), ("b_merge_ln", 6), ("bm_cnt", 7)):
                if flags.get(k):
                    t = wp.tile([P, P], F32, name=f"crow_{k}")
                    nc.sync.dma_start(t[:], crows_d[idx])
                    crow_s[k] = t
            b2_s = None
            if flags.get("bm") or flags.get("bm_cnt"):
                b2_s = wp.tile([2, P], F32)
                nc.sync.dma_start(b2_s[:], b2_d[:])
            ident = wp.tile([P, P], F32)
            make_identity(nc, ident[:])
            zero_sb = wp.tile([P, 512], F32)
            nc.vector.memset(zero_sb[:], 0.0)

            def load_idx(d, T, dt):
                t = wp.tile([P, T], dt, name=f"idx_{d.name}")
                nc.sync.dma_start(t[:], d[:])
                return t

            vg1_s = load_idx(vg1, T1, I32)
            cg1_s = load_idx(cg1, T1, I32)
            lid1_s = load_idx(lid1, T1, F32)
            uid1_s = load_idx(uid1, T1, I32)
            vg2_s = load_idx(vg2, T2, I32)
            cg2_s = load_idx(cg2, T2, I32)
            lid2_s = load_idx(lid2, T2, F32)
            uid2_s = load_idx(uid2, T2, I32)

            _memset_dram(nc, zero_sb, agg1[:], NVloc)
            _memset_dram(nc, zero_sb, agg2[:], NUloc)

            # --- row transform: out_dram <- LN(in_dram) @ W (+ bias row) ---
            def row_transform(src, n_rows, W_s, bias_key, out_dram, tag):
                nt = -(-n_rows // P)
                for t in range(nt):
                    r0 = t * P
                    rows = min(P, n_rows - r0)
                    x = sp.tile([P, 1, P], F32, tag=f"x{tag}")
                    nc.sync.dma_start(x[:rows, 0, :], src[r0:r0 + rows, :])
                    z = sp.tile([P, 1, P], F32, tag=f"z{tag}")
                    _ln_normalize(nc, sp, x[:rows], z[:rows], 1, rows, tag=tag)
                    pzt = pp.tile([P, P], F32, tag="zt")
                    nc.tensor.transpose(out=pzt[:, :rows], in_=z[:rows, 0, :],
                                        identity=ident[:rows, :rows])
                    zt = sp.tile([P, P], F32, tag=f"zt{tag}")
                    nc.vector.tensor_copy(zt[:, :rows], pzt[:, :rows])
                    pv = pp.tile([P, P], F32, tag="s")
                    nc.tensor.matmul(out=pv[:rows], lhsT=zt[:, :rows], rhs=W_s[:],
                                     start=True, stop=True)
                    o = sp.tile([P, P], F32, tag=f"o{tag}")
                    if bias_key and flags.get(bias_key):
                        nc.vector.tensor_tensor(o[:rows], pv[:rows],
                                                crow_s[bias_key][:rows],
                                                op=mybir.AluOpType.add)
                    else:
                        nc.scalar.copy(o[:rows], pv[:rows])
                    nc.sync.dma_start(out_dram[r0:r0 + rows, :], o[:rows])

            # --- phase 1: v_t, c_t1 + AllGather v_t ---
            row_transform(var_own, NUloc, Wl_s, "bl", vt_own[:], "p1v")
            row_transform(con_own, NVloc, Wr_s, "br", ct1_own[:], "p1c")
            nc.gpsimd.collective_compute(
                "AllGather", mybir.AluOpType.bypass, replica_groups=rg,
                ins=[vt_own.opt()], outs=[vt_full.opt()])

            # --- stage A: edge pipeline over groups of GRP tiles ---
            def stage_a(T, ze_d, vtab, vidx_s, ctab, cidx_s, lid_s, uid_s,
                        aggD, nloc, tag):
                zev = ze_d[:].rearrange("(t p) f -> p t f", p=P)
                bc_reg = nc.gpsimd.to_reg(nloc - 1)
                bc = mybir.AluOpType
                for gi in range(T // grp):
                    t0 = gi * grp
                    ze = sp.tile([P, grp, P], F32, tag="A_ze")
                    nc.sync.dma_start(ze[:], zev[:, t0:t0 + grp, :])
                    vgt = sp.tile([P, grp, P], F32, tag="A_vg")
                    cgt = sp.tile([P, grp, P], F32, tag="A_cg")
                    for g in range(grp):
                        nc.gpsimd.indirect_dma_start(
                            out=vgt[:, g, :], out_offset=None, in_=vtab,
                            in_offset=IndirectOffsetOnAxis(
                                ap=vidx_s[:, t0 + g:t0 + g + 1], axis=0))
                        nc.gpsimd.indirect_dma_start(
                            out=cgt[:, g, :], out_offset=None, in_=ctab,
                            in_offset=IndirectOffsetOnAxis(
                                ap=cidx_s[:, t0 + g:t0 + g + 1], axis=0))
                    # LN1 on raw edge rows
                    z = sp.tile([P, grp, P], F32, tag="A_z")
                    if "noLN" in _ablate:
                        z = ze
                    else:
                        _ln_normalize(nc, sp, ze[:], z[:], grp, tag="A1")
                    # transpose z -> zt
                    pzt = pp.tile([P, grp, P], F32, tag="zt")
                    for g in range(grp):
                        nc.tensor.transpose(out=pzt[:, g, :], in_=z[:, g, :],
                                            identity=ident[:])
                    zt = sp.tile([P, grp, P], F32, tag="A_zt")
                    nc.scalar.copy(zt[:], pzt[:])
                    # s = z @ We (+be) + vg + cg ; relu
                    ps = pp.tile([P, grp, P], F32, tag="s")
                    for g in range(grp):
                        nc.tensor.matmul(out=ps[:, g, :], lhsT=zt[:, g, :],
                                         rhs=We_s[:], start=True, stop=True)
                    s1 = sp.tile([P, grp, P], F32, tag="A_s1")
                    nc.vector.tensor_tensor(s1[:], ps[:], vgt[:], op=bc.add)
                    if flags.get("be"):
                        nc.vector.tensor_tensor(
                            s1[:], s1[:],
                            crow_s["be"][:].rearrange("p f -> p () f").to_broadcast([P, grp, P]),
                            op=bc.add)
                    r = sp.tile([P, grp, P], F32, tag="A_r")
                    nc.vector.tensor_tensor(r[:], s1[:], cgt[:], op=bc.add)
                    nc.scalar.activation(r[:], r[:],
                                         mybir.ActivationFunctionType.Relu)
                    # LN2
                    z2 = sp.tile([P, grp, P], F32, tag="A_z2")
                    if "noLN" in _ablate:
                        z2 = r
                    else:
                        _ln_normalize(nc, sp, r[:], z2[:], grp, tag="A2")
                    # j = z2 @ Wj (+bj)
                    pzt2 = pp.tile([P, grp, P], F32, tag="zt")
                    for g in range(grp):
                        nc.tensor.transpose(out=pzt2[:, g, :], in_=z2[:, g, :],
                                            identity=ident[:])
                    zt2 = sp.tile([P, grp, P], F32, tag="A_zt2")
                    nc.scalar.copy(zt2[:], pzt2[:])
                    pj = pp.tile([P, grp, P], F32, tag="j")
                    for g in range(grp):
                        nc.tensor.matmul(out=pj[:, g, :], lhsT=zt2[:, g, :],
                                         rhs=Wj_s[:], start=True, stop=True)
                    if flags.get("bj"):
                        jt = sp.tile([P, grp, P], F32, tag="A_jt")
                        nc.vector.tensor_tensor(
                            jt[:], pj[:],
                            crow_s["bj"][:].rearrange("p f -> p () f").to_broadcast([P, grp, P]),
                            op=bc.add)
                        j_ap = jt[:]
                    else:
                        j_ap = pj[:]
                    # LN3 (joint, affine folded into Wmb / cnt row)
                    z3 = sp.tile([P, grp, P], F32, tag="A_z3")
                    if "noLN" in _ablate:
                        nc.scalar.copy(z3[:], j_ap)
                    else:
                        _ln_normalize(nc, sp, j_ap, z3[:], grp, tag="A3")
                    # selection matrix S_T[p, u] = (lid[p] == u)
                    st_t = sp.tile([P, grp, P], F32, tag="A_st")
                    nc.vector.tensor_tensor(
                        st_t[:],
                        lid_s[:, t0:t0 + grp].rearrange("p g -> p g ()").to_broadcast([P, grp, P]),
                        iota_s[:].rearrange("p f -> p () f").to_broadcast([P, grp, P]),
                        op=bc.is_equal)
                    pagg = pp.tile([P, grp, P], F32, tag="agg")
                    for g in range(grp):
                        nc.tensor.matmul(out=pagg[:, g, :], lhsT=st_t[:, g, :],
                                         rhs=z3[:, g, :], start=True, stop=True)
                    aggs = sp.tile([P, grp, P], F32, tag="A_aggs")
                    nc.scalar.copy(aggs[:], pagg[:])
                    for g in range(grp):
                        nc.gpsimd.indirect_dma_start(
                            out=aggD, out_offset=IndirectOffsetOnAxis(
                                ap=uid_s[:, t0 + g:t0 + g + 1], axis=0),
                            in_=aggs[:, g, :], in_offset=None,
                            bounds_check=bc_reg, oob_is_err=False)

            stage_a(T1, ze1, vt_full[:], vg1_s, ct1_own[:], cg1_s,
                    lid1_s, uid1_s, agg1[:], NVloc, "1")

            # --- stage B: merge + residual over own destination rows ---
            def stage_b(n_rows, h_dram, aggD, cnt_d, cnt_key, out_dram,
                        ct2_out, tag):
                bc = mybir.AluOpType
                nt = -(-n_rows // P)
                for t in range(nt):
                    r0 = t * P
                    rows = min(P, n_rows - r0)
                    h = sp.tile([P, P], F32, tag="B_h")
                    nc.sync.dma_start(h[:rows], h_dram[r0:r0 + rows, :])
                    a = sp.tile([P, P], F32, tag="B_a")
                    nc.sync.dma_start(a[:rows], aggD[r0:r0 + rows, :])
                    pzt = pp.tile([P, P], F32, tag="zt")
                    nc.tensor.transpose(out=pzt[:, :rows], in_=h[:rows],
                                        identity=ident[:rows, :rows])
                    ht = sp.tile([P, P], F32, tag="B_ht")
                    nc.vector.tensor_copy(ht[:, :rows], pzt[:, :rows])
                    pzt2 = pp.tile([P, P], F32, tag="zt")
                    nc.tensor.transpose(out=pzt2[:, :rows], in_=a[:rows],
                                        identity=ident[:rows, :rows])
                    at = sp.tile([P, P], F32, tag="B_at")
                    nc.vector.tensor_copy(at[:, :rows], pzt2[:, :rows])
                    pm = pp.tile([P, P], F32, tag="s")
                    nc.tensor.matmul(out=pm[:rows], lhsT=ht[:, :rows],
                                     rhs=Wmt_s[:], start=True, stop=False)
                    use_cnt = flags.get(cnt_key)
                    nc.tensor.matmul(out=pm[:rows], lhsT=at[:, :rows],
                                     rhs=Wmb_s[:], start=False,
                                     stop=not (use_cnt or flags.get("bm")))
                    if use_cnt or flags.get("bm"):
                        onec = sp.tile([P, 2], F32, tag="B_onec")
                        if use_cnt:
                            nc.sync.dma_start(onec[:rows, 0:1],
                                              cnt_d[r0:r0 + rows, :])
                        else:
                            nc.vector.memset(onec[:rows, 0:1], 0.0)
                        nc.vector.memset(onec[:rows, 1:2], 1.0)
                        # pm += cnt * bm_cnt_row + 1 * bm_row  (rank-2 update)
                        pzt3 = pp.tile([P, P], F32, tag="zt")
                        nc.tensor.transpose(out=pzt3[:2, :rows], in_=onec[:rows],
                                            identity=ident[:rows, :rows])
                        oct_ = sp.tile([2, P], F32, tag="B_oct")
                        nc.vector.tensor_copy(oct_[:2, :rows], pzt3[:2, :rows])
                        nc.tensor.matmul(out=pm[:rows], lhsT=oct_[:2, :rows],
                                         rhs=b2_s[:2, :], start=False, stop=True)
                    rr = sp.tile([P, 1, P], F32, tag="B_r")
                    nc.scalar.activation(rr[:rows, 0, :], pm[:rows],
                                         mybir.ActivationFunctionType.Relu)
                    zm = sp.tile([P, 1, P], F32, tag="B_zm")
                    _ln_normalize(nc, sp, rr[:rows], zm[:rows], 1, rows, tag="B")
                    za = zm[:rows, 0, :]
                    if flags.get("g_merge"):
                        nc.vector.tensor_tensor(za, za, crow_s["g_merge"][:rows],
                                                op=bc.mult)
                    if flags.get("b_merge_ln"):
                        nc.vector.tensor_tensor(za, za,
                                                crow_s["b_merge_ln"][:rows],
                                                op=bc.add)
                    outt = sp.tile([P, P], F32, tag="B_out")
                    nc.vector.tensor_tensor(outt[:rows], za, h[:rows], op=bc.add)
                    nc.sync.dma_start(out_dram[r0:r0 + rows, :], outt[:rows])
                    if ct2_out is not None:
                        z4 = sp.tile([P, 1, P], F32, tag="B_z4")
                        _ln_normalize(nc, sp, outt[:rows].rearrange("p f -> p () f"),
                                      z4[:rows], 1, rows, tag="B2")
                        pzt4 = pp.tile([P, P], F32, tag="zt")
                        nc.tensor.transpose(out=pzt4[:, :rows],
                                            in_=z4[:rows, 0, :],
                                            identity=ident[:rows, :rows])
                        zt4 = sp.tile([P, P], F32, tag="B_zt4")
                        nc.vector.tensor_copy(zt4[:, :rows], pzt4[:, :rows])
                        pc2 = pp.tile([P, P], F32, tag="s")
                        nc.tensor.matmul(out=pc2[:rows], lhsT=zt4[:, :rows],
                                         rhs=Wr_s[:], start=True, stop=True)
                        o2 = sp.tile([P, P], F32, tag="B_o2")
                        if flags.get("br"):
                            nc.vector.tensor_tensor(o2[:rows], pc2[:rows],
                                                    crow_s["br"][:rows],
                                                    op=bc.add)
                        else:
                            nc.scalar.copy(o2[:rows], pc2[:rows])
                        nc.sync.dma_start(ct2_out[r0:r0 + rows, :], o2[:rows])

            if "noB" not in _ablate:
                stage_b(NVloc, ct1_own[:], agg1[:], cnt1_d, "bm_cnt1", new_con,
                        ct2_own[:], "1")

            nc.gpsimd.collective_compute(
                "AllGather", mybir.AluOpType.bypass, replica_groups=rg,
                ins=[ct2_own.opt()], outs=[ct2_full.opt()])

            if "noA2" not in _ablate:
                stage_a(T2, ze2, vt_own[:], vg2_s, ct2_full[:], cg2_s,
                        lid2_s, uid2_s, agg2[:], NUloc, "2")
            if "noB" not in _ablate:
                stage_b(NUloc, vt_own[:], agg2[:], cnt2_d, "bm_cnt2", new_var,
                        None, "2")

    _split_sync_waits(nc)
    return nc


# ---------------------------------------------------------------------------
# Entry point
# ---------------------------------------------------------------------------
_CACHE = {}


def _run(inputs, ncores=NCORES):
    i = {k: np.asarray(v) for k, v in inputs.items()}
    NU, EMB = i["variable_emb"].shape
    NV = i["constraint_emb"].shape[0]
    assert EMB == P
    w = _fold_weights(i)
    flags = {
        "bl": np.any(w["bl"] != 0), "be": np.any(w["be"] != 0),
        "br": np.any(w["br"] != 0), "bj": np.any(w["bj"] != 0),
        "bm": np.any(w["bm"] != 0),
        "bm_cnt1": np.any(w["bm_cnt"] != 0), "bm_cnt2": np.any(w["bm_cnt"] != 0),
        "bm_cnt": np.any(w["bm_cnt"] != 0),
        "g_merge": np.any(w["g_merge"] != 1.0),
        "b_merge_ln": np.any(w["b_merge_ln"] != 0),
    }
    e_u = i["e_u"].astype(np.int64)
    e_v = i["e_v"].astype(np.int64)
    import hashlib
    key = hashlib.sha1(
        e_u.tobytes() + e_v.tobytes()
        + repr(sorted(flags.items())).encode()
        + repr((NU, NV, ncores)).encode()).hexdigest()
    if key in _CACHE:
        p1, p2, nc = _CACHE[key]
    else:
        p1 = _prep_pass(e_v, e_u, NV, ncores, GRP)  # dest=constraints, gather v_t
        p2 = _prep_pass(e_u, e_v, NU, ncores, GRP)  # dest=variables, gather c_t2
        nc = build_program(NU, NV, p1["T"], p2["T"], flags)
        _CACHE.clear()
        _CACHE[key] = (p1, p2, nc)

    edge_emb = i["edge_emb"].astype(np.float32)
    NUloc, NVloc = NU // ncores, NV // ncores
    crows = np.zeros((8, P, P), np.float32)
    for idx, k in enumerate(["bl", "be", "br", "bj", "bm"]):
        crows[idx, :, :] = w[k][None, :]
    crows[5, :, :] = w["g_merge"][None, :]
    crows[6, :, :] = w["b_merge_ln"][None, :]
    crows[7, :, :] = w["bm_cnt"][None, :]
    iota128 = np.broadcast_to(np.arange(P, dtype=np.float32), (P, P)).copy()

    def ze_arr(prep, r):
        er = prep["e_rows"][r]
        out = np.zeros((er.shape[0], P), np.float32)
        m = er >= 0
        out[m] = edge_emb[er[m]]
        return out

    in_maps = []
    for r in range(ncores):
        in_maps.append({
            "var_own": i["variable_emb"][r * NUloc:(r + 1) * NUloc].astype(np.float32),
            "con_own": i["constraint_emb"][r * NVloc:(r + 1) * NVloc].astype(np.float32),
            "ze1": ze_arr(p1, r), "ze2": ze_arr(p2, r),
            "vg1": p1["g_other"][r], "cg1": p1["g_dloc"][r],
            "lid1": p1["lid"][r], "uid1": p1["uid"][r],
            "vg2": p2["g_dloc"][r], "cg2": p2["g_other"][r],
            "lid2": p2["lid"][r], "uid2": p2["uid"][r],
            "Wl": w["Wl"], "We": w["We"], "Wr": w["Wr"], "Wj": w["Wj"],
            "Wmt": w["Wmt"], "Wmb": w["Wmb"],
            "iota128": iota128, "crows": crows,
            "b2rows": np.stack([w["bm_cnt"], w["bm"]]),
            "cnt1": p1["cnts"][r][:, None], "cnt2": p2["cnts"][r][:, None],
        })

    res = run_bass_kernel_spmd(nc, in_maps, list(range(ncores)))
    new_var = np.concatenate([res.results[r]["new_var_own"] for r in range(ncores)])
    new_con = np.concatenate([res.results[r]["new_con_own"] for r in range(ncores)])
    return new_var, new_con


def kernel(**inputs):
    return _run(inputs)
